# revision 1
# baseline (speedup 1.0000x reference)
"""Causal self-attention (B=4, T=2048, C=1024, 16 heads) on 8 trn2 NeuronCores.

Sharding: tensor-parallel over heads (2 heads/core) for QKV + attention,
then an AllToAll reshards from head-split to token-split for the output
projection.  Each core returns out[token_slice, :]; the host concatenates.

Per-core pipeline (all cores run the identical program; only the fed
W_qkv column-slice differs):
  stage 1: qT,kT  [128ch x 2048tok]  and v [tok-major] per batch, from
           x @ W_qkv_slice  (x is DMA'd in transposed [c, t] tiles)
  stage 2: causal attention per (batch, head): S^T tiles [kt=128, q=512],
           exp on ACT (no max-subtraction: scores/8 ~ N(0,1), bounded),
           multiplicative 0/1 causal mask on diagonal blocks,
           AV accumulation with a ones-column appended to v so PSUM row 64
           carries the softmax denominators; normalize via reciprocal +
           partition_broadcast.
  stage 3: AllToAll (4.2MB/rank) -> y^T [1024ch, 1024tok] token slice,
           out = y^T.T @ W_proj  accumulated over 8 channel chunks.
"""

import os
import numpy as np

from concourse import bass, bacc, mybir, tile
from concourse.bass_utils import run_bass_kernel_spmd

F32 = mybir.dt.float32
F32R = mybir.dt.float32r
BF16 = mybir.dt.bfloat16

B, T, C = 4, 2048, 1024
H, D = 16, 64
NCORES = 8
HPC = H // NCORES            # heads per core = 2
QKC = HPC * D                # per-core q/k/v channels = 128
BT = B * T                   # 8192 tokens total
TPS = BT // NCORES           # tokens per core after A2A = 1024
P = 128
TW = 512                     # token window for stage1/attention q windows
NW = T // TW                 # windows per batch = 4
NKT = T // P                 # kt tiles per batch = 16

# ---- dtype knobs (bitcast matmul operands; float32r = full-rate PE) ----
RD = {"f32": F32, "f32r": F32R, "bf16": BF16}[os.environ.get("KMM_DT", "f32")]
PSS_BUFS = int(os.environ.get("KPSS", "4"))
PSY_BUFS = int(os.environ.get("KPSY", "2"))
DIAG_FIRST = os.environ.get("KDIAG", "1") == "1"
YU_COPY = os.environ.get("KYU", "1") == "1"


def _mm(ap, dt=None):
    return ap


def _causal_mask_01() -> np.ndarray:
    """mask[p, m, f] = 1.0 iff kt_local = 128*m + p <= f, for q windows of 512."""
    m = np.zeros((P, NW, TW), dtype=np.float32)
    p = np.arange(P)[:, None, None]
    mm = np.arange(NW)[None, :, None]
    f = np.arange(TW)[None, None, :]
    m[(P * mm + p) <= f] = 1.0
    return m


def build() -> bass.Bass:
    nc = bacc.Bacc(num_devices=NCORES, target_bir_lowering=False)

    x_d = nc.dram_tensor("x", [BT, C], F32, kind="ExternalInput")
    wqkv_d = nc.dram_tensor("wqkv", [C, 3 * QKC], F32, kind="ExternalInput")
    wproj_d = nc.dram_tensor("wproj", [C, C], F32, kind="ExternalInput")
    out_d = nc.dram_tensor("out", [TPS, C], F32, kind="ExternalOutput")

    mask_d = nc.inline_tensor(_causal_mask_01(), name="mask01")
    ident_d = nc.inline_tensor(np.eye(P, dtype=np.float32), name="ident")

    KC = C // P  # 8 contraction chunks

    with tile.TileContext(nc) as tc:
        from contextlib import ExitStack

        with ExitStack() as ctx:
            # ---- persistent pools ----
            wq_pool = ctx.enter_context(tc.tile_pool(name="wq", bufs=1))
            msk_pool = ctx.enter_context(tc.tile_pool(name="msk", bufs=1))
            wp_pool = ctx.enter_context(tc.tile_pool(name="wp", bufs=1))
            dram = ctx.enter_context(tc.tile_pool(name="dram", bufs=1, space="DRAM"))

            wqkv_sb = wq_pool.tile([P, KC, 3 * QKC], RD)
            mask_sb = msk_pool.tile([P, NW, TW], RD)
            ident_sb = msk_pool.tile([P, P], F32)
            nc.sync.dma_start(out=ident_sb[:], in_=ident_d[:, :])
            if RD is F32:
                ident_rd = ident_sb
            else:
                ident_rd = msk_pool.tile([P, P], RD)
                nc.vector.tensor_copy(ident_rd[:], ident_sb[:])
            wproj_sb = wp_pool.tile([P, KC, C], RD)
            if RD is F32:
                nc.sync.dma_start(
                    out=wqkv_sb[:], in_=x_dram_re(wqkv_d, "(k p) n -> p k n")
                )
                nc.sync.dma_start(out=mask_sb[:], in_=mask_d[:, :, :])
                nc.sync.dma_start(
                    out=wproj_sb[:], in_=x_dram_re(wproj_d, "(k p) n -> p k n")
                )
            else:
                with tc.tile_pool(name="stage", bufs=2) as stg:
                    wqkv_st = stg.tile([P, KC, 3 * QKC], F32, tag="st3", bufs=1)
                    nc.sync.dma_start(
                        out=wqkv_st[:], in_=x_dram_re(wqkv_d, "(k p) n -> p k n")
                    )
                    nc.vector.tensor_copy(wqkv_sb[:], wqkv_st[:])
                    mask_st = stg.tile([P, NW, TW], F32, tag="stm", bufs=1)
                    nc.sync.dma_start(out=mask_st[:], in_=mask_d[:, :, :])
                    nc.vector.tensor_copy(mask_sb[:], mask_st[:])
                    for kc in range(KC):
                        wproj_st = stg.tile([P, C], F32, tag="wst", name="wproj_st")
                        nc.sync.dma_start(
                            out=wproj_st[:],
                            in_=wproj_d[kc * P : (kc + 1) * P, :],
                        )
                        nc.vector.tensor_copy(wproj_sb[:, kc, :], wproj_st[:])

            y_send = dram.tile([NCORES, QKC, TPS], RD)
            y_recv = dram.tile([NCORES, QKC, TPS], RD)

            # ---- stage 1 + 2 pools ----
            s12 = ExitStack()
            with s12:
                xT_pool = s12.enter_context(tc.tile_pool(name="xT", bufs=2))
                qkv_pool = s12.enter_context(tc.tile_pool(name="qkv", bufs=2))
                ps1 = s12.enter_context(
                    tc.tile_pool(name="ps1", bufs=2, space="PSUM")
                )
                pss = s12.enter_context(
                    tc.tile_pool(name="pss", bufs=PSS_BUFS, space="PSUM")
                )
                psy = s12.enter_context(
                    tc.tile_pool(name="psy", bufs=PSY_BUFS, space="PSUM")
                )
                pt_pool = s12.enter_context(tc.tile_pool(name="pt", bufs=6))
                nrm_pool = s12.enter_context(tc.tile_pool(name="nrm", bufs=2))
                yt_pool = s12.enter_context(tc.tile_pool(name="yt", bufs=2))

                for b in range(B):
                    qT_b = qkv_pool.tile([P, T], RD, tag="qT")
                    kT_b = qkv_pool.tile([P, T], RD, tag="kT")
                    v_b = qkv_pool.tile([P, NKT, HPC, D + 1], RD, tag="v")
                    # ones column for softmax denominators
                    ones_col = v_b[:, :, :, D : D + 1]
                    if RD is F32R:
                        ones_col = ones_col.bitcast(F32)
                    nc.gpsimd.memset(ones_col, 1.0)

                    # ---- stage 1: qT, kT, v for batch b ----
                    for w in range(NW):
                        t0 = b * T + w * TW
                        # natural-layout x subtiles (contiguous 4KB-row DMAs)
                        xns = []
                        for s in range(TW // P):
                            xn = xT_pool.tile([P, C], F32, tag="xn", name="xn", bufs=6)
                            nc.sync.dma_start(
                                out=xn[:], in_=x_d[t0 + s * P : t0 + (s + 1) * P, :]
                            )
                            xns.append(xn)
                        # transpose to xT [c-part, tok] on the PE
                        xT = xT_pool.tile([P, KC, TW], RD)
                        for kc in range(KC):
                            ps_t = ps1.tile([P, TW], F32, tag="ps1", name="ps_t")
                            for s in range(TW // P):
                                nc.tensor.transpose(
                                    ps_t[:, s * P : (s + 1) * P],
                                    xns[s][:, kc * P : (kc + 1) * P],
                                    ident_sb[:],
                                )
                            nc.vector.tensor_copy(xT[:, kc, :], ps_t[:])
                        for which, dst in ((0, qT_b), (1, kT_b)):
                            ps = ps1.tile([P, TW], F32, tag="ps1")
                            for kc in range(KC):
                                nc.tensor.matmul(
                                    ps[:],
                                    lhsT=wqkv_sb[:, kc, which * QKC : (which + 1) * QKC],
                                    rhs=xT[:, kc, :],
                                    start=(kc == 0),
                                    stop=(kc == KC - 1),
                                )
                            nc.vector.tensor_copy(dst[:, w * TW : (w + 1) * TW], ps[:])
                        ps_vT = ps1.tile([P, TW], F32, tag="ps1", name="ps_vT")
                        for kc in range(KC):
                            nc.tensor.matmul(
                                ps_vT[:],
                                lhsT=wqkv_sb[:, kc, 2 * QKC : 3 * QKC],
                                rhs=xT[:, kc, :],
                                start=(kc == 0),
                                stop=(kc == KC - 1),
                            )
                        vT_sb = xT_pool.tile([P, TW], RD, tag="vT", name="vT_sb")
                        nc.vector.tensor_copy(vT_sb[:], ps_vT[:])
                        ps_v = ps1.tile([P, TW], RD, tag="ps1", name="ps_v")
                        for s in range(TW // P):
                            nc.tensor.transpose(
                                ps_v[:, s * P : (s + 1) * P],
                                vT_sb[:, s * P : (s + 1) * P],
                                ident_rd[:],
                            )
                        jt0 = w * (TW // P)
                        nc.vector.tensor_copy(
                            v_b[:, jt0 : jt0 + TW // P, :, 0:D],
                            ps_v[:].rearrange("p (s h d) -> p s h d", s=TW // P, h=HPC),
                        )

                    # ---- stage 2: attention for batch b ----
                    # h innermost: two independent AV chains overlap on PE
                    for w in range(NW):
                        for h in range(HPC):
                            qT_h = qT_b[h * D : (h + 1) * D, :]
                            kT_h = kT_b[h * D : (h + 1) * D, :]
                            nkt = (w + 1) * (TW // P)
                            ps_y = psy.tile([D + 1, TW], F32, tag="ps_y")
                            jks = list(range(nkt))
                            if DIAG_FIRST:
                                jks = jks[w * (TW // P):] + jks[: w * (TW // P)]
                            for ji, jk in enumerate(jks):
                                ps_s = pss.tile([P, TW], F32, tag="ps_s")
                                nc.tensor.matmul(
                                    ps_s[:],
                                    lhsT=kT_h[:, jk * P : (jk + 1) * P],
                                    rhs=qT_h[:, w * TW : (w + 1) * TW],
                                    start=True,
                                    stop=True,
                                )
                                pt = pt_pool.tile([P, TW], RD, tag="pt")
                                nc.scalar.activation(
                                    pt[:],
                                    ps_s[:],
                                    mybir.ActivationFunctionType.Exp,
                                    scale=1.0 / np.sqrt(D),
                                )
                                m = jk - w * (TW // P)
                                if m >= 0:
                                    nc.gpsimd.tensor_mul(
                                        pt[:], pt[:], mask_sb[:, m, :]
                                    )
                                nc.tensor.matmul(
                                    ps_y[:],
                                    lhsT=v_b[:, jk, h, :],
                                    rhs=pt[:],
                                    start=(ji == 0),
                                    stop=(ji == nkt - 1),
                                )
                            if YU_COPY:
                                yu = yt_pool.tile([D + 1, TW], F32, tag="yu", bufs=4)
                                nc.vector.tensor_copy(yu[:], ps_y[:])
                                src_y = yu
                            else:
                                src_y = ps_y
                            recip = nrm_pool.tile([1, TW], F32, tag="recip")
                            nc.vector.reciprocal(recip[:], src_y[D : D + 1, :])
                            bc = nrm_pool.tile([D, TW], F32, tag="bc")
                            nc.gpsimd.partition_broadcast(bc[:], recip[:])
                            yt = yt_pool.tile([D, TW], RD, tag="yt")
                            nc.vector.tensor_mul(yt[:], src_y[0:D, :], bc[:])
                            g0 = b * T + w * TW
                            shard = g0 // TPS
                            c0 = g0 % TPS
                            nc.sync.dma_start(
                                out=y_send[shard, h * D : (h + 1) * D, c0 : c0 + TW],
                                in_=yt[:],
                            )

            # ---- stage 3: A2A + projection ----
            nc.gpsimd.collective_compute(
                "AllToAll",
                mybir.AluOpType.bypass,
                replica_groups=[list(range(NCORES))],
                ins=[y_send.opt()],
                outs=[y_recv.opt()],
            )

            s3 = ExitStack()
            with s3:
                yr_pool = s3.enter_context(tc.tile_pool(name="yr", bufs=2))
                pso = s3.enter_context(tc.tile_pool(name="pso", bufs=2, space="PSUM"))
                ob_pool = s3.enter_context(tc.tile_pool(name="ob", bufs=2))
                for jt in range(TPS // P):
                    yr = yr_pool.tile([P, KC, P], RD, tag="yr")
                    nc.sync.dma_start(
                        out=yr[:],
                        in_=y_recv[:, :, jt * P : (jt + 1) * P].rearrange(
                            "k p t -> p k t"
                        ),
                    )
                    for half in range(C // TW):
                        ps_o = pso.tile([P, TW], F32, tag="ps_o")
                        for kc in range(KC):
                            nc.tensor.matmul(
                                ps_o[:],
                                lhsT=yr[:, kc, :],
                                rhs=wproj_sb[:, kc, half * TW : (half + 1) * TW],
                                start=(kc == 0),
                                stop=(kc == KC - 1),
                            )
                        ob = ob_pool.tile([P, TW], F32, tag="ob")
                        nc.vector.tensor_copy(ob[:], ps_o[:])
                        nc.sync.dma_start(
                            out=out_d[jt * P : (jt + 1) * P, half * TW : (half + 1) * TW],
                            in_=ob[:],
                        )

    nc.finalize()
    return nc


def x_dram_re(handle, pattern):
    return handle[:, :].rearrange(pattern, p=P)


_NC_CACHE: dict = {}


def _get_nc() -> bass.Bass:
    if "nc" not in _NC_CACHE:
        _NC_CACHE["nc"] = build()
    return _NC_CACHE["nc"]


def shard_inputs(x, W_qkv, W_proj):
    x = np.ascontiguousarray(np.asarray(x, dtype=np.float32).reshape(BT, C))
    W_qkv = np.asarray(W_qkv, dtype=np.float32)
    W_proj = np.ascontiguousarray(np.asarray(W_proj, dtype=np.float32))
    in_maps = []
    for c in range(NCORES):
        cols = slice(QKC * c, QKC * (c + 1))
        w_c = np.ascontiguousarray(
            np.concatenate(
                [W_qkv[:, cols], W_qkv[:, C:][:, cols], W_qkv[:, 2 * C :][:, cols]],
                axis=1,
            )
        )
        in_maps.append({"x": x, "wqkv": w_c, "wproj": W_proj})
    return in_maps


def run(in_maps, trace=False, **kwargs):
    return run_bass_kernel_spmd(
        _get_nc(), in_maps, core_ids=list(range(NCORES)), trace=trace, **kwargs
    )


def kernel(x, W_qkv, W_proj):
    res = run(shard_inputs(x, W_qkv, W_proj), trace=False)
    out = np.concatenate([res.results[c]["out"] for c in range(NCORES)], axis=0)
    return out.reshape(B, T, C).astype(np.float32)



# revision 6
# speedup vs baseline: 1.4379x; 1.4379x over previous
"""Causal self-attention (B=4, T=2048, C=1024, 16 heads) on 8 trn2 NeuronCores.

Sharding: tensor-parallel over heads (2 heads/core) for QKV + attention,
then per-batch AllToAll reshards from head-split to token-split for the
output projection.  Output tokens are striped: core c owns, for every
batch b, tokens [b*2048 + c*256, b*2048 + (c+1)*256).  The host gather
interleaves them back.

All matmul operands are float32r (full-rate single-pass PE).  The BIR
verifier requires every f32r matmul input to be produced by a rounding
instruction, so tensors feeding matmuls are written by DVE copies /
ACT activations with f32r dst (DMA+bitcast is rejected).

Per-core pipeline (identical program on all cores; only the fed W_qkv
column-slice differs):
  per batch b, per 512-token window w (stage1+stage2 interleaved):
    stage1: xn [tok,1024] tiles DMA'd naturally; PE-transposed (plain
            f32) to xT [c,tok] (DVE evac rounds to f32r); qT,kT
            [128ch x tok] and v [tok-major] from x @ W_qkv_slice.
    stage2: causal attention per head: S^T tiles [kt=128, q=512],
            exp on ACT -> f32r pt (scores/8 ~ N(0,1), no max
            subtraction), multiplicative 0/1 mask on diagonal blocks
            (DVE), AV accumulation with a ones-column in v so PSUM row
            64 carries softmax denominators; normalize via 1/d =
            exp(-ln d) on ACT + gpsimd partition_broadcast + DVE mul.
  after batch b: AllToAll #b (1MB/rank) -> yT [1024ch, 256tok] chunk;
  proj for chunk b-1 runs under batch b's compute (1-batch software
  pipeline); only A2A #3 + proj #3 are exposed at the tail.

Engine assignment keeps serial chains off congested engines:
  PE: transposes + all matmuls.  ACT: exp, yu/ob evac, ln/exp
  reciprocal.  DVE: xT/qk/v evacuations, diag masks, normalize mul,
  memset.  GpSimd: partition_broadcast + collective trigger only
  (avoids ucode library thrash).
"""

import os
import numpy as np

from concourse import bass, bacc, mybir, tile
from concourse.bass_utils import run_bass_kernel_spmd

F32 = mybir.dt.float32
F32R = mybir.dt.float32r

B, T, C = 4, 2048, 1024
H, D = 16, 64
NCORES = 8
HPC = H // NCORES            # heads per core = 2
QKC = HPC * D                # per-core q/k/v channels = 128
BT = B * T                   # 8192 tokens total
TPS = BT // NCORES           # tokens per core after A2A = 1024
CHK = TPS // B               # tokens per (core, batch) chunk = 256
P = 128
TW = 512                     # q window
NW = T // TW                 # windows per batch = 4
NKT = T // P                 # kt tiles per batch = 16
KC = C // P                  # contraction chunks = 8

DIAG_FIRST = os.environ.get("KDIAG", "1") == "1"


def _causal_mask_01() -> np.ndarray:
    """mask[p, m, f] = 1.0 iff kt_local = 128*m + p <= f, for q windows of 512."""
    m = np.zeros((P, NW, TW), dtype=np.float32)
    p = np.arange(P)[:, None, None]
    mm = np.arange(NW)[None, :, None]
    f = np.arange(TW)[None, None, :]
    m[(P * mm + p) <= f] = 1.0
    return m


def build() -> bass.Bass:
    nc = bacc.Bacc(num_devices=NCORES, target_bir_lowering=False)

    x_d = nc.dram_tensor("x", [BT, C], F32, kind="ExternalInput")
    wqkv_d = nc.dram_tensor("wqkv", [C, 3 * QKC], F32, kind="ExternalInput")
    wproj_d = nc.dram_tensor("wproj", [C, C], F32, kind="ExternalInput")
    out_d = nc.dram_tensor("out", [TPS, C], F32, kind="ExternalOutput")

    mask_d = nc.inline_tensor(_causal_mask_01(), name="mask01")
    ident_d = nc.inline_tensor(np.eye(P, dtype=np.float32), name="ident")

    with tile.TileContext(nc) as tc:
        from contextlib import ExitStack

        with ExitStack() as ctx:
            # ---- persistent pools ----
            wq_pool = ctx.enter_context(tc.tile_pool(name="wq", bufs=1))
            dram = ctx.enter_context(tc.tile_pool(name="dram", bufs=1, space="DRAM"))

            wqkv_sb = wq_pool.tile([P, KC, 3 * QKC], F32R, tag="wqkv")
            mask_sb = wq_pool.tile([P, NW, TW], F32R, tag="mask")
            ident_sb = wq_pool.tile([P, P], F32, tag="ident")
            ident_rd = wq_pool.tile([P, P], F32R, tag="identr")
            wproj_sb = wq_pool.tile([P, KC, C], F32R, tag="wproj")
            nc.sync.dma_start(out=ident_sb[:], in_=ident_d[:, :])
            nc.vector.tensor_copy(ident_rd[:], ident_sb[:])
            with tc.tile_pool(name="stage", bufs=2) as stg:
                wqkv_st = stg.tile([P, KC, 3 * QKC], F32, tag="st3", bufs=1)
                nc.sync.dma_start(
                    out=wqkv_st[:],
                    in_=wqkv_d[:, :].rearrange("(k p) n -> p k n", p=P),
                )
                nc.vector.tensor_copy(wqkv_sb[:], wqkv_st[:])
                mask_st = stg.tile([P, NW, TW], F32, tag="stm", bufs=1)
                nc.sync.dma_start(out=mask_st[:], in_=mask_d[:, :, :])
                nc.vector.tensor_copy(mask_sb[:], mask_st[:])
                for kc in range(KC):
                    wproj_st = stg.tile([P, C], F32, tag="wst", name="wproj_st")
                    nc.sync.dma_start(
                        out=wproj_st[:], in_=wproj_d[kc * P : (kc + 1) * P, :]
                    )
                    nc.vector.tensor_copy(wproj_sb[:, kc, :], wproj_st[:])

            y_send = [
                dram.tile([NCORES, QKC, CHK], F32, tag=f"ys{b}", name=f"y_send{b}")
                for b in range(B)
            ]
            y_recv = [
                dram.tile([NCORES, QKC, CHK], F32, tag=f"yr{b}", name=f"y_recv{b}")
                for b in range(B)
            ]

            # ---- working pools ----
            xT_pool = ctx.enter_context(tc.tile_pool(name="xT", bufs=2))
            qkv_pool = ctx.enter_context(tc.tile_pool(name="qkv", bufs=2))
            ps1 = ctx.enter_context(tc.tile_pool(name="ps1", bufs=2, space="PSUM"))
            pss = ctx.enter_context(tc.tile_pool(name="pss", bufs=2, space="PSUM"))
            psy = ctx.enter_context(tc.tile_pool(name="psy", bufs=2, space="PSUM"))
            pso = ctx.enter_context(tc.tile_pool(name="pso", bufs=2, space="PSUM"))
            pt_pool = ctx.enter_context(tc.tile_pool(name="pt", bufs=4))
            nrm_pool = ctx.enter_context(tc.tile_pool(name="nrm", bufs=2))
            yt_pool = ctx.enter_context(tc.tile_pool(name="yt", bufs=2))
            yr_pool = ctx.enter_context(tc.tile_pool(name="yr", bufs=2))
            ob_pool = ctx.enter_context(tc.tile_pool(name="ob", bufs=2))

            def stage3(b):
                yr = yr_pool.tile([P, NCORES, CHK], F32, tag="yr", bufs=1)
                nc.sync.dma_start(
                    out=yr[:], in_=y_recv[b][:, :, :].rearrange("k p t -> p k t")
                )
                yr_r = yr_pool.tile([P, NCORES, CHK], F32R, tag="yrr", bufs=1)
                nc.vector.tensor_copy(yr_r[:], yr[:])
                for sub in range(CHK // P):
                    for half in range(C // TW):
                        ps_o = pso.tile([P, TW], F32, tag="ps_o")
                        for kc in range(KC):
                            nc.tensor.matmul(
                                ps_o[:],
                                lhsT=yr_r[:, kc, sub * P : (sub + 1) * P],
                                rhs=wproj_sb[:, kc, half * TW : (half + 1) * TW],
                                start=(kc == 0),
                                stop=(kc == KC - 1),
                            )
                        ob = ob_pool.tile([P, TW], F32, tag="ob")
                        nc.scalar.copy(ob[:], ps_o[:])
                        nc.sync.dma_start(
                            out=out_d[
                                b * CHK + sub * P : b * CHK + (sub + 1) * P,
                                half * TW : (half + 1) * TW,
                            ],
                            in_=ob[:],
                        )

            for b in range(B):
                qT_b = qkv_pool.tile([P, T], F32R, tag="qT")
                kT_b = qkv_pool.tile([P, T], F32R, tag="kT")
                v_b = qkv_pool.tile([P, NKT, HPC, D + 1], F32R, tag="v")
                # ones column for softmax denominators
                nc.vector.memset(v_b[:, :, :, D : D + 1].bitcast(F32), 1.0)

                for w in range(NW):
                    t0 = b * T + w * TW
                    # ---- stage 1: qT, kT, v for window w ----
                    xns = []
                    for s in range(TW // P):
                        xn = xT_pool.tile([P, C], F32, tag="xn", name="xn", bufs=4)
                        nc.sync.dma_start(
                            out=xn[:], in_=x_d[t0 + s * P : t0 + (s + 1) * P, :]
                        )
                        xns.append(xn)
                    xT = xT_pool.tile([P, KC, TW], F32R, tag="xT")
                    for kc in range(KC):
                        ps_t = ps1.tile([P, TW], F32, tag="ps1", name="ps_t")
                        for s in range(TW // P):
                            nc.tensor.transpose(
                                ps_t[:, s * P : (s + 1) * P],
                                xns[s][:, kc * P : (kc + 1) * P],
                                ident_sb[:],
                            )
                        nc.vector.tensor_copy(xT[:, kc, :], ps_t[:])
                    for which, dst in ((0, qT_b), (1, kT_b)):
                        ps = ps1.tile([P, TW], F32, tag="ps1")
                        for kc in range(KC):
                            nc.tensor.matmul(
                                ps[:],
                                lhsT=wqkv_sb[:, kc, which * QKC : (which + 1) * QKC],
                                rhs=xT[:, kc, :],
                                start=(kc == 0),
                                stop=(kc == KC - 1),
                            )
                        nc.vector.tensor_copy(dst[:, w * TW : (w + 1) * TW], ps[:])
                    ps_vT = ps1.tile([P, TW], F32, tag="ps1", name="ps_vT")
                    for kc in range(KC):
                        nc.tensor.matmul(
                            ps_vT[:],
                            lhsT=wqkv_sb[:, kc, 2 * QKC : 3 * QKC],
                            rhs=xT[:, kc, :],
                            start=(kc == 0),
                            stop=(kc == KC - 1),
                        )
                    vT_sb = xT_pool.tile([P, TW], F32R, tag="vT", name="vT_sb")
                    nc.vector.tensor_copy(vT_sb[:], ps_vT[:])
                    ps_v = ps1.tile([P, TW], F32R, tag="ps1", name="ps_v")
                    for s in range(TW // P):
                        nc.tensor.transpose(
                            ps_v[:, s * P : (s + 1) * P],
                            vT_sb[:, s * P : (s + 1) * P],
                            ident_rd[:],
                        )
                    jt0 = w * (TW // P)
                    nc.vector.tensor_copy(
                        v_b[:, jt0 : jt0 + TW // P, :, 0:D],
                        ps_v[:].rearrange("p (s h d) -> p s h d", s=TW // P, h=HPC),
                    )

                    # ---- stage 2: attention for window w ----
                    for h in range(HPC):
                        qT_h = qT_b[h * D : (h + 1) * D, :]
                        kT_h = kT_b[h * D : (h + 1) * D, :]
                        nkt = (w + 1) * (TW // P)
                        ps_y = psy.tile([D + 1, TW], F32, tag="ps_y")
                        jks = list(range(nkt))
                        if DIAG_FIRST:
                            jks = jks[w * (TW // P) :] + jks[: w * (TW // P)]
                        for ji, jk in enumerate(jks):
                            ps_s = pss.tile([P, TW], F32, tag="ps_s")
                            nc.tensor.matmul(
                                ps_s[:],
                                lhsT=kT_h[:, jk * P : (jk + 1) * P],
                                rhs=qT_h[:, w * TW : (w + 1) * TW],
                                start=True,
                                stop=True,
                            )
                            pt = pt_pool.tile([P, TW], F32R, tag="pt")
                            nc.scalar.activation(
                                pt[:],
                                ps_s[:],
                                mybir.ActivationFunctionType.Exp,
                                scale=1.0 / np.sqrt(D),
                            )
                            m = jk - w * (TW // P)
                            if m >= 0:
                                nc.vector.tensor_mul(pt[:], pt[:], mask_sb[:, m, :])
                            nc.tensor.matmul(
                                ps_y[:],
                                lhsT=v_b[:, jk, h, :],
                                rhs=pt[:],
                                start=(ji == 0),
                                stop=(ji == nkt - 1),
                            )
                        yu = yt_pool.tile([D + 1, TW], F32, tag="yu", bufs=4)
                        nc.scalar.copy(yu[:], ps_y[:])
                        # 1/d = exp(-ln d) on ACT (vector.reciprocal is ~3.3us)
                        lnd = nrm_pool.tile([1, TW], F32, tag="lnd")
                        nc.scalar.activation(
                            lnd[:], yu[D : D + 1, :], mybir.ActivationFunctionType.Ln
                        )
                        recip = nrm_pool.tile([1, TW], F32, tag="recip")
                        nc.scalar.activation(
                            recip[:],
                            lnd[:],
                            mybir.ActivationFunctionType.Exp,
                            scale=-1.0,
                        )
                        bc = nrm_pool.tile([D, TW], F32, tag="bc")
                        nc.gpsimd.partition_broadcast(bc[:], recip[:])
                        yt = yt_pool.tile([D, TW], F32, tag="yt")
                        nc.vector.tensor_mul(yt[:], yu[0:D, :], bc[:])
                        for half in range(TW // CHK):
                            s = (TW // CHK) * w + half
                            nc.sync.dma_start(
                                out=y_send[b][s, h * D : (h + 1) * D, :],
                                in_=yt[:, half * CHK : (half + 1) * CHK],
                            )

                # ---- A2A for batch b; proj for batch b-1 under batch b+1 ----
                nc.gpsimd.collective_compute(
                    "AllToAll",
                    mybir.AluOpType.bypass,
                    replica_groups=[list(range(NCORES))],
                    ins=[y_send[b].opt()],
                    outs=[y_recv[b].opt()],
                )
                if b > 0:
                    stage3(b - 1)
            stage3(B - 1)

    nc.finalize()
    return nc


_NC_CACHE: dict = {}


def _get_nc() -> bass.Bass:
    if "nc" not in _NC_CACHE:
        _NC_CACHE["nc"] = build()
    return _NC_CACHE["nc"]


def shard_inputs(x, W_qkv, W_proj):
    x = np.ascontiguousarray(np.asarray(x, dtype=np.float32).reshape(BT, C))
    W_qkv = np.asarray(W_qkv, dtype=np.float32)
    W_proj = np.ascontiguousarray(np.asarray(W_proj, dtype=np.float32))
    in_maps = []
    for c in range(NCORES):
        cols = slice(QKC * c, QKC * (c + 1))
        w_c = np.ascontiguousarray(
            np.concatenate(
                [W_qkv[:, cols], W_qkv[:, C:][:, cols], W_qkv[:, 2 * C :][:, cols]],
                axis=1,
            )
        )
        in_maps.append({"x": x, "wqkv": w_c, "wproj": W_proj})
    return in_maps


def run(in_maps, trace=False, **kwargs):
    return run_bass_kernel_spmd(
        _get_nc(), in_maps, core_ids=list(range(NCORES)), trace=trace, **kwargs
    )


def gather(res) -> np.ndarray:
    """Un-stripe: core c's out rows are [b*CHK, (b+1)*CHK) = batch b tokens
    [b*T + c*CHK, b*T + (c+1)*CHK)."""
    outs = np.stack([res.results[c]["out"] for c in range(NCORES)])  # [8, TPS, C]
    full = outs.reshape(NCORES, B, CHK, C).transpose(1, 0, 2, 3).reshape(B, T, C)
    return np.ascontiguousarray(full)


def kernel(x, W_qkv, W_proj):
    res = run(shard_inputs(x, W_qkv, W_proj), trace=False)
    return gather(res).astype(np.float32)
